# revision 1
# baseline (speedup 1.0000x reference)
"""FCOS detection head on 8 Trainium2 NeuronCores (Bass/Tile).

Data parallel: batch 16 -> 2 images per core. Weights replicated.

Per-core compute layout:
  - channels on SBUF partitions (256 ch -> 2 tiles of 128), spatial flattened
    on the free dim, activations stored zero-padded (H+2)x(W+2) so a 3x3 conv
    is 2(ci) x 9(taps) = 18 accumulating matmuls per PSUM tile.
  - matmuls run as float32r (full-rate fp32 path on the PE).
  - bias+ReLU epilogues on the scalar engine (ACT) straight out of PSUM into
    the next layer's padded buffer.
  - head outputs (85 = 80 cls + 4 box + 1 ctr channels) are assembled
    channels-on-partitions, then PE-transposed per 128-position chunk into
    (positions, 85) and DMA'd to HBM.
"""

import numpy as np

import concourse.bacc as bacc
import concourse.bass as bass
import concourse.mybir as mybir
import concourse.tile as tile
from concourse.bass import ts
from concourse.bass_utils import run_bass_kernel_spmd
from concourse.masks import make_identity

F32 = mybir.dt.float32
N_CORES = 8
B_FULL = 16
BS = B_FULL // N_CORES  # images per core
C = 256
NCLS = 80
SDEPTH = 4
TAPS = [(ky, kx) for ky in range(3) for kx in range(3)]

# (H, W, rows-per-block for direct convs / Winograd stem, output base offset);
# p5 runs both images per layer and uses direct stem (spatial too small for
# N=512 Winograd matmuls)
LEVELS = [
    dict(H=64, W=64, R=8, Rw=16, base=0, img_groups=[[0], [1]]),
    dict(H=32, W=32, R=16, Rw=32, base=4096, img_groups=[[0], [1]]),
    dict(H=16, W=16, R=16, Rw=None, base=5120, img_groups=[[0, 1]]),
]
HW_TOTAL = 64 * 64 + 32 * 32 + 16 * 16  # 5376

# matmul/storage dtype for conv operands: float16 runs the PE at full rate
# (1 row/cycle, like bf16) with 10 mantissa bits; PSUM accumulation is fp32.
# (fp32 matmul is 4x slower; fp32r's fused weight-load path caps at 2 sync
# waits per instruction, which Tile-scheduled code exceeds.)
F16 = mybir.dt.float16


def _conv_block(nc, psum, src, wslices, y0, R, W, start_clear=True):
    """18 accumulating matmuls: psum[M, R*W] += sum_{ci,tap} w.T @ x_shifted.

    src: padded activation tile [128, 2, H+2, W+2]
    wslices: wslices[ci][tap] -> lhsT AP [128, M]
    """
    n_ci = len(wslices)
    for ci in range(n_ci):
        for t, (dy, dx) in enumerate(TAPS):
            rhs = src[:, ci, y0 + dy : y0 + dy + R, dx : dx + W]
            nc.tensor.matmul(
                psum[:],
                wslices[ci][t],
                rhs,
                start=(start_clear and ci == 0 and t == 0),
                stop=(ci == n_ci - 1 and t == len(TAPS) - 1),
            )


def _border_memset(nc, buf, H, W):
    # zero the 1-px padding border of a [128, 2, H+2, W+2] tile
    nc.gpsimd.memset(buf[:, :, 0, :], 0.0)
    nc.gpsimd.memset(buf[:, :, H + 1, :], 0.0)
    nc.gpsimd.memset(buf[:, :, 1 : H + 1, 0], 0.0)
    nc.gpsimd.memset(buf[:, :, 1 : H + 1, W + 1], 0.0)


def build_nc():
    # Bacc so finalize() runs the wait-legalization passes (matmul waits
    # move to ldweights / event-semaphore splits) that walrus codegen needs.
    nc = bacc.Bacc()

    # --- DRAM parameters (per-core views) ---
    x_dram = {}
    for i, lvl in enumerate(LEVELS):
        H, W = lvl["H"], lvl["W"]
        # host-side zero-padded to (H+2, W+2): one fully contiguous DMA per
        # (image, ci-tile) and no on-chip border memsets for the x buffer
        x_dram[i] = nc.declare_dram_parameter(
            f"x_l{i}", [BS, C, H + 2, W + 2], F16, isOutput=False
        )
    w_cls = nc.declare_dram_parameter("w_cls", [SDEPTH, 2, 128, 2 * 9 * 128], F16, isOutput=False)
    w_box = nc.declare_dram_parameter("w_box", [SDEPTH, 2, 128, 2 * 9 * 128], F16, isOutput=False)
    # Winograd F(2,3)-transformed stem weights: cols = (co_t, i4, dy3, co128)
    w_cls_w = nc.declare_dram_parameter("w_cls_w", [SDEPTH, 2, 128, 2 * 4 * 3 * 128], F16, isOutput=False)
    w_box_w = nc.declare_dram_parameter("w_box_w", [SDEPTH, 2, 128, 2 * 4 * 3 * 128], F16, isOutput=False)
    w_pcls = nc.declare_dram_parameter("w_pcls", [2, 128, 9 * NCLS], F16, isOutput=False)
    w_pbc = nc.declare_dram_parameter("w_pbc", [2, 128, 9 * 5], F16, isOutput=False)
    b_stem = nc.declare_dram_parameter("b_stem", [128, 2 * SDEPTH * 2], F32, isOutput=False)
    b_pred = nc.declare_dram_parameter("b_pred", [85, 1], F32, isOutput=False)
    out_dram = nc.declare_dram_parameter("out", [BS, HW_TOTAL, 85], F32, isOutput=True)

    with tile.TileContext(nc) as tc:
        with (
            tc.tile_pool(name="const", bufs=1) as const,
            tc.tile_pool(name="wp", bufs=3) as wp,
            tc.tile_pool(name="acts", bufs=1) as acts,
            tc.tile_pool(name="stage", bufs=1) as stage,
            tc.tile_pool(name="pp", bufs=1, space="PSUM") as pp,
        ):
            # constants
            ident = const.tile([128, 128], F32, name="ident")
            make_identity(nc, ident[:])
            bst = const.tile([128, 2, SDEPTH, 2, 1], F32, name="bst")
            nc.sync.dma_start(out=bst[:, :, :, :, 0], in_=b_stem[:].rearrange("p (t l c) -> p t l c", t=2, l=SDEPTH, c=2))
            bp_cls = const.tile([NCLS, 1], F32, name="bp_cls")
            nc.sync.dma_start(out=bp_cls[:], in_=b_pred[0:NCLS])
            bp_bc = const.tile([5, 1], F32, name="bp_bc")
            nc.sync.dma_start(out=bp_bc[:], in_=b_pred[NCLS : NCLS + 5])
            wpc = const.tile([128, 2, 9 * NCLS], F16, name="wpc")
            wpb = const.tile([128, 2, 9 * 5], F16, name="wpb")
            for t in range(2):
                nc.sync.dma_start(out=wpc[:, t, :], in_=w_pcls[t])
                nc.sync.dma_start(out=wpb[:, t, :], in_=w_pbc[t])

            for li, lvl in enumerate(LEVELS):
                H, W, R, base = lvl["H"], lvl["W"], lvl["R"], lvl["base"]
                HP, WP = H + 2, W + 2
                nblk = H // R
                N = R * W  # psum free size per block

                for imgs in lvl["img_groups"]:
                    # padded activation buffers per image: x, A, B
                    xb, ab, bb, sbc = {}, {}, {}, {}
                    for slot, g in enumerate(imgs):
                        xb[g] = acts.tile([128, 2, HP, WP], F16, name=f"xb{slot}", tag=f"xb{slot}")
                        ab[g] = acts.tile([128, 2, HP, WP], F16, name=f"ab{slot}", tag=f"ab{slot}")
                        bb[g] = acts.tile([128, 2, HP, WP], F16, name=f"bb{slot}", tag=f"bb{slot}")
                        sbc[g] = stage.tile([NCLS, H * W], F32, name=f"sbc{slot}", tag=f"sbc{slot}")
                        for buf in (ab[g], bb[g]):
                            _border_memset(nc, buf, H, W)
                        hh = HP // 2
                        for t in range(2):
                            nc.sync.dma_start(
                                out=xb[g][:, t, 0:hh, :],
                                in_=x_dram[li][g, ts(t, 128), 0:hh],
                            )
                            nc.sync.dma_start(
                                out=xb[g][:, t, hh:HP, :],
                                in_=x_dram[li][g, ts(t, 128), hh:HP],
                            )

                    Rw = lvl["Rw"]
                    Wh = W // 2

                    def stem_layer_direct(tower_w, tower_idx, lay, src_of, dst_of):
                        wt = wp.tile([128, 2, 2 * 9 * 128], F16, name="wt", tag="wt")
                        for t in range(2):
                            nc.sync.dma_start(out=wt[:, t, :], in_=tower_w[lay, t])
                        for g in imgs:
                            src, dst = src_of[g], dst_of[g]
                            for blk in range(nblk):
                                y0 = blk * R
                                for co in range(2):
                                    ps = pp.tile([128, N], F32, name="ps", tag="ws0", bufs=2)
                                    wsl_co = [
                                        [wt[:, ci, ts(co * 9 + t, 128)] for t in range(9)]
                                        for ci in range(2)
                                    ]
                                    _conv_block(nc, ps, src, wsl_co, y0, R, W)
                                    nc.scalar.activation(
                                        dst[:, co, 1 + y0 : 1 + y0 + R, 1 : 1 + W],
                                        ps[:].rearrange("p (r w) -> p r w", w=W),
                                        mybir.ActivationFunctionType.Relu,
                                        bias=bst[:, tower_idx, lay, co, :],
                                    )

                    def stem_layer_wino(tower_w, tower_idx, lay, src_of, dst_of):
                        """1D Winograd F(2,3) along W: 24 matmuls of N=Rw*W/2
                        per (block, co) instead of 18 of N=Rw*W/... (1.5x fewer
                        PE rows). Input/output transforms run on the DVE."""
                        wt = wp.tile([128, 2, 2 * 4 * 3 * 128], F16, name="wtw", tag="wt")
                        for t in range(2):
                            nc.sync.dma_start(out=wt[:, t, :], in_=tower_w[lay, t])
                        add = mybir.AluOpType.add
                        sub = mybir.AluOpType.subtract
                        for g in imgs:
                            src, dst = src_of[g], dst_of[g]
                            for blk in range(H // Rw):
                                y0 = blk * Rw
                                # one tile per transform index so matmul group i
                                # only waits on its own V op (Tile deps are
                                # whole-tile); i-major emission gets the PE
                                # started after the first two ops.
                                vt = [
                                    stage.tile(
                                        [128, 2, Rw + 2, Wh], F16, name=f"vt{i}", tag=f"vt{i}", bufs=3
                                    )
                                    for i in range(4)
                                ]
                                vdef = [
                                    (0, 0, 2, sub),
                                    (1, 1, 2, add),
                                    (2, 2, 1, sub),
                                    (3, 1, 3, sub),
                                ]
                                for i, a, b, op in vdef:
                                    for ci in range(2):
                                        rows = src[:, ci, y0 : y0 + Rw + 2, :]
                                        nc.vector.tensor_tensor(
                                            vt[i][:, ci],
                                            rows[:, :, a : a + W - 1 : 2],
                                            rows[:, :, b : b + W - 1 : 2],
                                            op,
                                        )
                                for co in range(2):
                                    ps = [
                                        pp.tile([128, Rw, Wh], F32, name=f"ws{i}", tag=f"ws{i}", bufs=2)
                                        for i in range(4)
                                    ]
                                    for i in range(4):
                                        for dy in range(3):
                                            for ci in range(2):
                                                nc.tensor.matmul(
                                                    ps[i][:],
                                                    wt[:, ci, ts((co * 4 + i) * 3 + dy, 128)],
                                                    vt[i][:, ci, dy : dy + Rw, :],
                                                    start=(dy == 0 and ci == 0),
                                                    stop=(dy == 2 and ci == 1),
                                                )
                                    # DVE may read only ONE PSUM operand per op:
                                    # stage m2 in SBUF (on ACT), then combine on
                                    # DVE with one PSUM operand per instruction.
                                    c2 = stage.tile([128, Rw, Wh], F32, name="c2", tag="c2", bufs=2)
                                    t0 = stage.tile([128, Rw, Wh], F32, name="t0", tag="t0", bufs=2)
                                    e0 = stage.tile([128, Rw, Wh], F32, name="e0", tag="e0", bufs=2)
                                    e1 = stage.tile([128, Rw, Wh], F32, name="e1", tag="e1", bufs=2)
                                    nc.scalar.activation(
                                        c2[:], ps[2][:], mybir.ActivationFunctionType.Copy
                                    )
                                    nc.vector.tensor_tensor(t0[:], ps[1][:], c2[:], add)
                                    nc.vector.tensor_tensor(e0[:], ps[0][:], t0[:], add)
                                    nc.vector.tensor_tensor(e1[:], ps[1][:], c2[:], sub)
                                    nc.vector.tensor_tensor(e1[:], e1[:], ps[3][:], sub)
                                    nc.scalar.activation(
                                        dst[:, co, 1 + y0 : 1 + y0 + Rw, 1 : W + 1 : 2],
                                        e0[:],
                                        mybir.ActivationFunctionType.Relu,
                                        bias=bst[:, tower_idx, lay, co, :],
                                    )
                                    nc.scalar.activation(
                                        dst[:, co, 1 + y0 : 1 + y0 + Rw, 2 : W + 2 : 2],
                                        e1[:],
                                        mybir.ActivationFunctionType.Relu,
                                        bias=bst[:, tower_idx, lay, co, :],
                                    )

                    def stem_layer(tower_dir, tower_wino, tower_idx, lay, src_of, dst_of):
                        if Rw is None:
                            stem_layer_direct(tower_dir, tower_idx, lay, src_of, dst_of)
                        else:
                            stem_layer_wino(tower_wino, tower_idx, lay, src_of, dst_of)

                    # --- cls tower: x->A->B->A->B ---
                    ping = {0: xb, 1: ab, 2: bb, 3: ab}
                    pong = {0: ab, 1: bb, 2: ab, 3: bb}
                    for lay in range(SDEPTH):
                        stem_layer(w_cls, w_cls_w, 0, lay, ping[lay], pong[lay])

                    # --- cls pred: B -> sb_cls (bias, no relu) ---
                    wsl_pc = [[wpc[:, ci, ts(t, NCLS)] for t in range(9)] for ci in range(2)]
                    for g in imgs:
                        for blk in range(nblk):
                            y0 = blk * R
                            psc = pp.tile([NCLS, N], F32, name="psc", tag="ws1", bufs=2)
                            _conv_block(nc, psc, bb[g], wsl_pc, y0, R, W)
                            nc.scalar.activation(
                                sbc[g][:, y0 * W : y0 * W + N],
                                psc[:],
                                mybir.ActivationFunctionType.Identity,
                                bias=bp_cls[:],
                            )

                    # --- box tower: x->A->x->A->x ---
                    bping = {0: xb, 1: ab, 2: xb, 3: ab}
                    bpong = {0: ab, 1: xb, 2: ab, 3: xb}
                    for lay in range(SDEPTH):
                        stem_layer(w_box, w_box_w, 1, lay, bping[lay], bpong[lay])

                    # --- box+ctr pred from x; assemble + write output ---
                    wsl_pb = [[wpb[:, ci, ts(t, 5)] for t in range(9)] for ci in range(2)]
                    for g in imgs:
                        for blk in range(nblk):
                            y0 = blk * R
                            psb = pp.tile([5, N], F32, name="psb", tag="ws2", bufs=2)
                            _conv_block(nc, psb, xb[g], wsl_pb, y0, R, W)
                            sbb = stage.tile([5, N], F32, name="sbb", tag="sbb", bufs=2)
                            nc.scalar.activation(
                                sbb[:],
                                psb[:],
                                mybir.ActivationFunctionType.Identity,
                                bias=bp_bc[:],
                            )
                            for c0 in range(0, N, 128):
                                s0 = y0 * W + c0
                                pst = pp.tile([128, 85], F32, name="pst", tag="ws3", bufs=2)
                                nc.tensor.transpose(
                                    pst[:, 0:NCLS],
                                    sbc[g][:, s0 : s0 + 128],
                                    ident[0:NCLS, 0:NCLS],
                                )
                                nc.tensor.transpose(
                                    pst[:, NCLS:85],
                                    sbb[:, c0 : c0 + 128],
                                    ident[0:5, 0:5],
                                )
                                osb = stage.tile([128, 85], F32, name="osb", tag="osb", bufs=4)
                                nc.scalar.activation(
                                    osb[:], pst[:], mybir.ActivationFunctionType.Copy
                                )
                                nc.sync.dma_start(
                                    out=out_dram[g, base + s0 : base + s0 + 128, :],
                                    in_=osb[:],
                                )
    return nc


def prep_weights(inputs):
    """Host-side reshape of conv weights into lhsT ([ci, co] per tap) layouts."""

    def stem(w):  # (S, O=256, I=256, 3, 3) -> (S, ci_t 2, ci 128, co_t*tap*co)
        S = w.shape[0]
        t = w.transpose(0, 2, 3, 4, 1)  # (S, I, ky, kx, O)
        t = t.reshape(S, 2, 128, 9, 2, 128)  # (S, ci_t, ci, tap, co_t, co)
        t = t.transpose(0, 1, 2, 4, 3, 5)  # (S, ci_t, ci, co_t, tap, co)
        return np.ascontiguousarray(t.reshape(S, 2, 128, 2 * 9 * 128))

    def pred(w):  # (O, 256, 3, 3) -> (ci_t 2, ci 128, tap*O)
        O = w.shape[0]
        t = w.transpose(1, 2, 3, 0)  # (I, ky, kx, O)
        t = t.reshape(2, 128, 9, O)
        return np.ascontiguousarray(t.reshape(2, 128, 9 * O))

    def stem_wino(w):  # (S, O, I, 3, 3) -> (S, ci_t, ci, (co_t i4 dy3 co128))
        S = w.shape[0]
        G = np.array([[1, 0, 0], [0.5, 0.5, 0.5], [0.5, -0.5, 0.5], [0, 0, 1]], np.float64)
        U = np.einsum("xk,soidk->soixd", G, w.astype(np.float64))  # (S,O,I,4,3)
        t = U.transpose(0, 2, 3, 4, 1)  # (S, I, i4, dy3, O)
        t = t.reshape(S, 2, 128, 4, 3, 2, 128)  # (S, ci_t, ci, i, dy, co_t, co)
        t = t.transpose(0, 1, 2, 5, 3, 4, 6)  # (S, ci_t, ci, co_t, i, dy, co)
        return np.ascontiguousarray(t.reshape(S, 2, 128, 2 * 4 * 3 * 128))

    wm = {}
    wm["w_cls"] = stem(inputs["stem_cls_w"]).astype(np.float16)
    wm["w_box"] = stem(inputs["stem_box_w"]).astype(np.float16)
    wm["w_cls_w"] = stem_wino(inputs["stem_cls_w"]).astype(np.float16)
    wm["w_box_w"] = stem_wino(inputs["stem_box_w"]).astype(np.float16)
    wm["w_pcls"] = pred(inputs["pred_cls_w"]).astype(np.float16)
    wm["w_pbc"] = pred(
        np.concatenate([inputs["pred_box_w"], inputs["pred_ctr_w"]], axis=0)
    ).astype(np.float16)
    # stem biases: (S, 256) per tower -> [128, (tower, layer, co_t)]
    bs = np.stack([inputs["stem_cls_b"], inputs["stem_box_b"]], axis=0)  # (2, S, 256)
    bs = bs.reshape(2, SDEPTH, 2, 128).transpose(3, 0, 1, 2)  # (128, 2, S, 2)
    wm["b_stem"] = np.ascontiguousarray(bs.reshape(128, 2 * SDEPTH * 2))
    wm["b_pred"] = np.concatenate(
        [inputs["pred_cls_b"], inputs["pred_box_b"], inputs["pred_ctr_b"]]
    ).reshape(85, 1)
    return {
        k: v if v.dtype == np.float16 else v.astype(np.float32) for k, v in wm.items()
    }


_NC_CACHE = None


def _get_nc():
    global _NC_CACHE
    if _NC_CACHE is None:
        _NC_CACHE = build_nc()
    return _NC_CACHE


def run(inputs, **spmd_kwargs):
    inputs = {k: np.asarray(v) for k, v in inputs.items()}
    nc = _get_nc()
    if not nc.is_finalized():
        nc.finalize()
    wm = prep_weights(inputs)
    feats = [inputs["feat_p3"], inputs["feat_p4"], inputs["feat_p5"]]
    in_maps = []
    for core in range(N_CORES):
        m = dict(wm)
        sl = slice(core * BS, (core + 1) * BS)
        for li in range(3):
            f = feats[li][sl]
            fp = np.zeros(
                (f.shape[0], f.shape[1], f.shape[2] + 2, f.shape[3] + 2), np.float16
            )
            fp[:, :, 1:-1, 1:-1] = f
            m[f"x_l{li}"] = fp
        in_maps.append(m)
    res = run_bass_kernel_spmd(nc, in_maps, list(range(N_CORES)), **spmd_kwargs)
    out = np.concatenate([res.results[i]["out"] for i in range(N_CORES)], axis=0)
    return out, res


def kernel(**inputs):
    return run(inputs)[0]



# revision 3
# speedup vs baseline: 1.3553x; 1.3553x over previous
"""FCOS detection head on 8 Trainium2 NeuronCores (Bass/Tile), fp8 DoubleRow.

Data parallel: batch 16 -> 2 images per core. Weights replicated.

Per-core compute layout:
  - channels on SBUF partitions (256 ch -> DoubleRow-fused pair of 128-ch
    k-tiles), spatial flattened on the free dim, activations stored fp8(e4m3)
    zero-padded (H+2)x(W+2) in a 16B-aligned pitch so a 3x3 conv is 9
    DoubleRow matmuls (or 12 for the 1D F(2,3) Winograd form) per PSUM tile.
  - weights are scaled by 512 on the host before fp8 quantization; the
    epilogue activation applies 1/512.
  - stem layers run either 1D Winograd F(2,3) along W (input transform on the
    DVE, output transform DVE+ACT) or direct 3x3 (no DVE work) -- the mix is
    chosen per tower/layer to balance PE vs DVE load.
  - head outputs (85 = 80 cls + 4 box + 1 ctr channels) are assembled
    channels-on-partitions, then PE-transposed per 128-position chunk into
    (positions, 85) and DMA'd to HBM.
"""

import numpy as np
import ml_dtypes

import concourse.bacc as bacc
import concourse.bass as bass
import concourse.mybir as mybir
import concourse.tile as tile
from concourse.bass import ts
from concourse.bass_utils import run_bass_kernel_spmd
from concourse.masks import make_identity

F32 = mybir.dt.float32
F8 = mybir.dt.float8e4
NP8 = ml_dtypes.float8_e4m3
DRM = mybir.MatmulPerfMode.DoubleRow
WSCALE = 512.0
INV_WSCALE = float(1.0 / WSCALE)

N_CORES = 8
B_FULL = 16
BS = B_FULL // N_CORES  # images per core
C = 256
NCLS = 80
NBC = 16  # box+ctr head padded to 16 output channels (5 real)
SDEPTH = 4
TAPS = [(ky, kx) for ky in range(3) for kx in range(3)]

# Winograd/direct mode per (level, tower, layer). 'w' = F(2,3) wino, 'd' = direct.
# Chosen to balance PE (direct-heavy) vs DVE (wino-heavy).
WMODE = [
    ("wwww", "wwdd"),  # p3 (cls, box)
    ("wwww", "wwdd"),  # p4
    ("dddd", "dddd"),  # p5
]

# (H, W, direct rows/blk, wino rows/blk, output base, padded width)
LEVELS = [
    dict(H=64, W=64, R=8, Rw=16, base=0, WPa=72, seq_imgs=True),
    dict(H=32, W=32, R=16, Rw=32, base=4096, WPa=40, seq_imgs=True),
    dict(H=16, W=16, R=16, Rw=None, base=5120, WPa=24, seq_imgs=False),
]
HW_TOTAL = 64 * 64 + 32 * 32 + 16 * 16  # 5376


def build_nc():
    nc = bacc.Bacc()

    x_dram = {}
    for i, lvl in enumerate(LEVELS):
        H, WPa = lvl["H"], lvl["WPa"]
        x_dram[i] = nc.declare_dram_parameter(
            f"x_l{i}", [BS, C, H + 2, WPa], F8, isOutput=False
        )
    # direct stem weights: (S, ci 128, ci_t 2, co_t*tap*co)
    w_cls_d = nc.declare_dram_parameter("w_cls_d", [SDEPTH, 128, 2, 2 * 9 * 128], F8, isOutput=False)
    w_box_d = nc.declare_dram_parameter("w_box_d", [SDEPTH, 128, 2, 2 * 9 * 128], F8, isOutput=False)
    # Winograd F(2,3)-transformed stem weights: (S, ci, ci_t, co_t*i4*dy3*co)
    w_cls_w = nc.declare_dram_parameter("w_cls_w", [SDEPTH, 128, 2, 2 * 4 * 3 * 128], F8, isOutput=False)
    w_box_w = nc.declare_dram_parameter("w_box_w", [SDEPTH, 128, 2, 2 * 4 * 3 * 128], F8, isOutput=False)
    w_pcls = nc.declare_dram_parameter("w_pcls", [128, 2, 9 * NCLS], F8, isOutput=False)
    w_pbc = nc.declare_dram_parameter("w_pbc", [128, 2, 9 * NBC], F8, isOutput=False)
    b_stem = nc.declare_dram_parameter("b_stem", [128, 2 * SDEPTH * 2], F32, isOutput=False)
    b_pcls = nc.declare_dram_parameter("b_pcls", [NCLS, 1], F32, isOutput=False)
    b_pbc = nc.declare_dram_parameter("b_pbc", [NBC, 1], F32, isOutput=False)
    out_dram = nc.declare_dram_parameter("out", [BS, HW_TOTAL, 85], F32, isOutput=True)

    add = mybir.AluOpType.add
    sub = mybir.AluOpType.subtract

    with tile.TileContext(nc) as tc:
        with (
            tc.tile_pool(name="const", bufs=1) as const,
            tc.tile_pool(name="wp", bufs=3) as wp,
            tc.tile_pool(name="acts", bufs=1) as acts,
            tc.tile_pool(name="stage", bufs=1) as stage,
            tc.tile_pool(name="pp", bufs=1, space="PSUM") as pp,
        ):
            ident = const.tile([128, 128], F32, name="ident")
            make_identity(nc, ident[:])
            bst = const.tile([128, 2, SDEPTH, 2, 1], F32, name="bst")
            nc.sync.dma_start(out=bst[:, :, :, :, 0], in_=b_stem[:].rearrange("p (t l c) -> p t l c", t=2, l=SDEPTH, c=2))
            bp_cls = const.tile([NCLS, 1], F32, name="bp_cls")
            nc.sync.dma_start(out=bp_cls[:], in_=b_pcls[:])
            bp_bc = const.tile([NBC, 1], F32, name="bp_bc")
            nc.sync.dma_start(out=bp_bc[:], in_=b_pbc[:])
            wpc = const.tile([128, 2, 9 * NCLS], F8, name="wpc")
            nc.sync.dma_start(out=wpc[:], in_=w_pcls[:])
            wpb = const.tile([128, 2, 9 * NBC], F8, name="wpb")
            nc.sync.dma_start(out=wpb[:], in_=w_pbc[:])

            for li, lvl in enumerate(LEVELS):
                H, W, R, Rw, base, WPa = (
                    lvl["H"], lvl["W"], lvl["R"], lvl["Rw"], lvl["base"], lvl["WPa"]
                )
                HP = H + 2
                nblk = H // R
                N = R * W  # direct psum free size
                Wh = W // 2
                cls_mode, box_mode = WMODE[li]
                img_groups = [[0], [1]] if lvl["seq_imgs"] else [[0, 1]]

                def border_memset(buf):
                    nc.gpsimd.memset(buf[:, :, 0, :], 0.0)
                    nc.gpsimd.memset(buf[:, :, H + 1, :], 0.0)
                    nc.gpsimd.memset(buf[:, :, 1 : H + 1, 0], 0.0)
                    nc.gpsimd.memset(buf[:, :, 1 : H + 1, W + 1], 0.0)

                for imgs in img_groups:
                    xb, a1, b1, a2, sbc = {}, {}, {}, {}, {}
                    for slot, g in enumerate(imgs):
                        xb[g] = acts.tile([128, 2, HP, WPa], F8, name=f"xb{slot}", tag=f"xb{slot}", bufs=2)
                        a1[g] = acts.tile([128, 2, HP, WPa], F8, name=f"a1{slot}", tag=f"a1{slot}")
                        b1[g] = acts.tile([128, 2, HP, WPa], F8, name=f"b1{slot}", tag=f"b1{slot}")
                        a2[g] = acts.tile([128, 2, HP, WPa], F8, name=f"a2{slot}", tag=f"a2{slot}")
                        sbc[g] = stage.tile([NCLS, H * W], F32, name=f"sbc{slot}", tag=f"sbc{slot}")
                        for buf in (a1[g], b1[g], a2[g]):
                            border_memset(buf)
                        hh = HP // 2
                        for t in range(2):
                            nc.sync.dma_start(
                                out=xb[g][:, t, 0:hh, :],
                                in_=x_dram[li][g, ts(t, 128), 0:hh],
                            )
                            nc.sync.dma_start(
                                out=xb[g][:, t, hh:HP, :],
                                in_=x_dram[li][g, ts(t, 128), hh:HP],
                            )

                    # ---- layer helpers ----
                    def load_w(dram, lay, wino):
                        if wino:
                            wt = wp.tile([128, 2, 2 * 4 * 3 * 128], F8, name="wtw", tag="wtw")
                        else:
                            wt = wp.tile([128, 2, 2 * 9 * 128], F8, name="wtd", tag="wtd")
                        nc.sync.dma_start(out=wt[:], in_=dram[lay])
                        return wt

                    def make_vt(src, y0, nrows):
                        """4 F(2,3) input-transform values for rows y0..y0+nrows-1."""
                        vt = [
                            stage.tile([128, 2, nrows, Wh], F8, name=f"vt{i}", tag=f"vt{i}", bufs=3)
                            for i in range(4)
                        ]
                        rows = src[:, :, y0 : y0 + nrows, :]
                        Ej = rows[:, :, :, 0 : W : 2]
                        Ej1 = rows[:, :, :, 2 : W + 2 : 2]
                        Oj = rows[:, :, :, 1 : W + 1 : 2]
                        Oj1 = rows[:, :, :, 3 : W + 3 : 2]
                        nc.vector.tensor_tensor(vt[0][:], Ej, Ej1, sub)
                        nc.vector.tensor_tensor(vt[1][:], Oj, Ej1, add)
                        nc.vector.tensor_tensor(vt[2][:], Ej1, Oj, sub)
                        nc.vector.tensor_tensor(vt[3][:], Oj, Oj1, sub)
                        return vt

                    def wino_mms_epilogue(wt, vt, dst, tower_idx, lay, y0):
                        """12 DR matmuls + output transform for one (block, both co)."""
                        for co in range(2):
                            ps = [
                                pp.tile([128, Rw, Wh], F32, name=f"ws{i}", tag=f"ws{i}", bufs=2)
                                for i in range(4)
                            ]
                            for i in range(4):
                                for dy in range(3):
                                    nc.tensor.matmul(
                                        ps[i][:],
                                        wt[:, :, ts((co * 4 + i) * 3 + dy, 128)],
                                        vt[i][:, :, dy : dy + Rw, :],
                                        start=(dy == 0),
                                        stop=(dy == 2),
                                        perf_mode=DRM,
                                    )
                            c2 = stage.tile([128, Rw, Wh], F32, name="c2", tag="c2", bufs=2)
                            t0 = stage.tile([128, Rw, Wh], F32, name="t0", tag="t0", bufs=2)
                            e0 = stage.tile([128, Rw, Wh], F32, name="e0", tag="e0", bufs=2)
                            e1 = stage.tile([128, Rw, Wh], F32, name="e1", tag="e1", bufs=2)
                            nc.scalar.activation(c2[:], ps[2][:], mybir.ActivationFunctionType.Copy)
                            nc.vector.tensor_tensor(t0[:], ps[1][:], c2[:], add)
                            nc.vector.tensor_tensor(e0[:], ps[0][:], t0[:], add)
                            nc.vector.tensor_tensor(e1[:], ps[1][:], c2[:], sub)
                            nc.vector.tensor_tensor(e1[:], e1[:], ps[3][:], sub)
                            nc.scalar.activation(
                                dst[:, co, 1 + y0 : 1 + y0 + Rw, 1 : W + 1 : 2],
                                e0[:],
                                mybir.ActivationFunctionType.Relu,
                                bias=bst[:, tower_idx, lay, co, :],
                                scale=INV_WSCALE,
                            )
                            nc.scalar.activation(
                                dst[:, co, 1 + y0 : 1 + y0 + Rw, 2 : W + 2 : 2],
                                e1[:],
                                mybir.ActivationFunctionType.Relu,
                                bias=bst[:, tower_idx, lay, co, :],
                                scale=INV_WSCALE,
                            )

                    def direct_layer(wt, src, dst, tower_idx, lay, g):
                        for blk in range(nblk):
                            y0 = blk * R
                            for co in range(2):
                                psd = pp.tile([128, N], F32, name="psd", tag=f"ws{co * 2 + (blk % 2)}", bufs=2)
                                for t, (dy, dx) in enumerate(TAPS):
                                    nc.tensor.matmul(
                                        psd[:],
                                        wt[:, :, ts(co * 9 + t, 128)],
                                        src[:, :, y0 + dy : y0 + dy + R, dx : dx + W],
                                        start=(t == 0),
                                        stop=(t == 8),
                                        perf_mode=DRM,
                                    )
                                nc.scalar.activation(
                                    dst[:, co, 1 + y0 : 1 + y0 + R, 1 : 1 + W],
                                    psd[:].rearrange("p (r w) -> p r w", w=W),
                                    mybir.ActivationFunctionType.Relu,
                                    bias=bst[:, tower_idx, lay, co, :],
                                    scale=INV_WSCALE,
                                )

                    # ---- stem towers, interleaved per layer ----
                    # cls ping-pong: x -> a1 -> b1 -> a1 -> b1 (pred reads b1)
                    # box ping-pong: x -> a2 -> x  -> a2 -> x  (pred reads x)
                    cls_pp = [(xb, a1), (a1, b1), (b1, a1), (a1, b1)]
                    box_pp = [(xb, a2), (a2, xb), (xb, a2), (a2, xb)]

                    for lay in range(SDEPTH):
                        cw = cls_mode[lay] == "w"
                        bw = box_mode[lay] == "w"
                        wtc = load_w(w_cls_w if cw else w_cls_d, lay, cw)
                        wtb = load_w(w_box_w if bw else w_box_d, lay, bw)
                        share_vt = lay == 0 and cw and bw
                        for g in imgs:
                            csrc, cdst = cls_pp[lay][0][g], cls_pp[lay][1][g]
                            bsrc, bdst = box_pp[lay][0][g], box_pp[lay][1][g]
                            if share_vt:
                                for blk in range(H // Rw):
                                    y0 = blk * Rw
                                    vt = make_vt(csrc, y0, Rw + 2)
                                    wino_mms_epilogue(wtc, vt, cdst, 0, lay, y0)
                                    wino_mms_epilogue(wtb, vt, bdst, 1, lay, y0)
                            else:
                                if cw:
                                    for blk in range(H // Rw):
                                        y0 = blk * Rw
                                        vt = make_vt(csrc, y0, Rw + 2)
                                        wino_mms_epilogue(wtc, vt, cdst, 0, lay, y0)
                                else:
                                    direct_layer(wtc, csrc, cdst, 0, lay, g)
                                if bw:
                                    for blk in range(H // Rw):
                                        y0 = blk * Rw
                                        vt = make_vt(bsrc, y0, Rw + 2)
                                        wino_mms_epilogue(wtb, vt, bdst, 1, lay, y0)
                                else:
                                    direct_layer(wtb, bsrc, bdst, 1, lay, g)

                    # ---- cls pred: b1 -> sbc (bias, no relu) ----
                    for g in imgs:
                        for blk in range(nblk):
                            y0 = blk * R
                            psc = pp.tile([NCLS, N], F32, name="psc", tag="ws1", bufs=2)
                            for t, (dy, dx) in enumerate(TAPS):
                                nc.tensor.matmul(
                                    psc[:],
                                    wpc[:, :, ts(t, NCLS)],
                                    b1[g][:, :, y0 + dy : y0 + dy + R, dx : dx + W],
                                    start=(t == 0),
                                    stop=(t == 8),
                                    perf_mode=DRM,
                                )
                            nc.scalar.activation(
                                sbc[g][:, y0 * W : y0 * W + N],
                                psc[:],
                                mybir.ActivationFunctionType.Identity,
                                bias=bp_cls[:],
                                scale=INV_WSCALE,
                            )

                    # ---- box+ctr pred from xb; assemble + write output ----
                    for g in imgs:
                        for blk in range(nblk):
                            y0 = blk * R
                            psb = pp.tile([NBC, N], F32, name="psb", tag="ws2", bufs=2)
                            for t, (dy, dx) in enumerate(TAPS):
                                nc.tensor.matmul(
                                    psb[:],
                                    wpb[:, :, ts(t, NBC)],
                                    xb[g][:, :, y0 + dy : y0 + dy + R, dx : dx + W],
                                    start=(t == 0),
                                    stop=(t == 8),
                                    perf_mode=DRM,
                                )
                            sbb = stage.tile([NBC, N], F32, name="sbb", tag="sbb", bufs=2)
                            nc.scalar.activation(
                                sbb[:],
                                psb[:],
                                mybir.ActivationFunctionType.Identity,
                                bias=bp_bc[:],
                                scale=INV_WSCALE,
                            )
                            for c0 in range(0, N, 128):
                                s0 = y0 * W + c0
                                pst = pp.tile([128, 85], F32, name="pst", tag="ws3", bufs=2)
                                nc.tensor.transpose(
                                    pst[:, 0:NCLS],
                                    sbc[g][:, s0 : s0 + 128],
                                    ident[0:NCLS, 0:NCLS],
                                )
                                nc.tensor.transpose(
                                    pst[:, NCLS:85],
                                    sbb[0:5, c0 : c0 + 128],
                                    ident[0:5, 0:5],
                                )
                                osb = stage.tile([128, 85], F32, name="osb", tag="osb", bufs=4)
                                nc.scalar.activation(
                                    osb[:], pst[:], mybir.ActivationFunctionType.Copy
                                )
                                nc.sync.dma_start(
                                    out=out_dram[g, base + s0 : base + s0 + 128, :],
                                    in_=osb[:],
                                )
    return nc


def q8(x):
    return np.clip(x, -240.0, 240.0).astype(NP8)


def prep_weights(inputs):
    """Host-side reshape + fp8 quantization of conv weights into DR lhsT layouts."""
    G = np.array(
        [[1, 0, 0], [0.5, 0.5, 0.5], [0.5, -0.5, 0.5], [0, 0, 1]], np.float64
    )

    def stem_direct(w):  # (S, O, I, 3, 3) -> (S, ci 128, cit 2, cot*tap*co)
        S = w.shape[0]
        t = w.reshape(S, 2, 128, 2, 128, 3, 3)  # (s, cot, co, cit, ci, ky, kx)
        t = t.transpose(0, 4, 3, 1, 5, 6, 2)  # (s, ci, cit, cot, ky, kx, co)
        return np.ascontiguousarray(t.reshape(S, 128, 2, 2 * 9 * 128))

    def stem_wino(w):  # (S, O, I, 3, 3) -> (S, ci, cit, cot*i4*dy3*co)
        S = w.shape[0]
        U = np.einsum("xk,soidk->soixd", G, w.astype(np.float64))  # (S,O,I,4,3)
        t = U.reshape(S, 2, 128, 2, 128, 4, 3)  # (s, cot, co, cit, ci, x, dy)
        t = t.transpose(0, 4, 3, 1, 5, 6, 2)  # (s, ci, cit, cot, x, dy, co)
        return np.ascontiguousarray(t.reshape(S, 128, 2, 2 * 4 * 3 * 128))

    def pred(w, opad):  # (O, 256, 3, 3) -> (ci 128, cit 2, tap*opad)
        O = w.shape[0]
        wp_ = np.zeros((opad, C, 3, 3), np.float64)
        wp_[:O] = w
        t = wp_.reshape(opad, 2, 128, 3, 3)  # (o, cit, ci, ky, kx)
        t = t.transpose(2, 1, 3, 4, 0)  # (ci, cit, ky, kx, o)
        return np.ascontiguousarray(t.reshape(128, 2, 9 * opad))

    wm = {}
    wm["w_cls_d"] = q8(stem_direct(inputs["stem_cls_w"]) * WSCALE)
    wm["w_box_d"] = q8(stem_direct(inputs["stem_box_w"]) * WSCALE)
    wm["w_cls_w"] = q8(stem_wino(inputs["stem_cls_w"]) * WSCALE)
    wm["w_box_w"] = q8(stem_wino(inputs["stem_box_w"]) * WSCALE)
    wm["w_pcls"] = q8(pred(inputs["pred_cls_w"], NCLS) * WSCALE)
    wm["w_pbc"] = q8(
        pred(
            np.concatenate([inputs["pred_box_w"], inputs["pred_ctr_w"]], axis=0), NBC
        )
        * WSCALE
    )
    # stem biases: (S, 256) per tower -> [128, (tower, layer, co_t)]
    bs = np.stack([inputs["stem_cls_b"], inputs["stem_box_b"]], axis=0)  # (2, S, 256)
    bs = bs.reshape(2, SDEPTH, 2, 128).transpose(3, 0, 1, 2)  # (128, 2, S, 2)
    wm["b_stem"] = np.ascontiguousarray(bs.reshape(128, 2 * SDEPTH * 2)).astype(np.float32)
    wm["b_pcls"] = inputs["pred_cls_b"].reshape(NCLS, 1).astype(np.float32)
    bbc = np.zeros((NBC, 1), np.float32)
    bbc[0:4, 0] = inputs["pred_box_b"]
    bbc[4, 0] = inputs["pred_ctr_b"][0]
    wm["b_pbc"] = bbc
    return wm


_NC_CACHE = None


def _get_nc():
    global _NC_CACHE
    if _NC_CACHE is None:
        _NC_CACHE = build_nc()
    return _NC_CACHE


def run(inputs, **spmd_kwargs):
    inputs = {k: np.asarray(v) for k, v in inputs.items()}
    nc = _get_nc()
    if not nc.is_finalized():
        nc.finalize()
    wm = prep_weights(inputs)
    feats = [inputs["feat_p3"], inputs["feat_p4"], inputs["feat_p5"]]
    in_maps = []
    for core in range(N_CORES):
        m = dict(wm)
        sl = slice(core * BS, (core + 1) * BS)
        for li, lvl in enumerate(LEVELS):
            f = feats[li][sl]
            fp = np.zeros((f.shape[0], f.shape[1], f.shape[2] + 2, lvl["WPa"]), NP8)
            fp[:, :, 1:-1, 1 : 1 + f.shape[3]] = q8(f)
            m[f"x_l{li}"] = fp
        in_maps.append(m)
    res = run_bass_kernel_spmd(nc, in_maps, list(range(N_CORES)), **spmd_kwargs)
    out = np.concatenate([res.results[i]["out"] for i in range(N_CORES)], axis=0)
    return out, res


def kernel(**inputs):
    return run(inputs)[0]
